# revision 15
# baseline (speedup 1.0000x reference)
"""Trainium2 Bass kernel for CombinedRepeatCausalLinear (parallel forward).

Computes out[b,e,t] = sum_s x[b,e,s] * W[s,t] + bias[t] where
  W[s,t] = mask(t>=s) * (w0[s]*d0^(t-s) + w1[t]*d1^(t-s))
for S = 2048, x of shape (8, 1024, 2048) fp32.

Strategy (8 NeuronCores, data-parallel over batch; fp16 datapath):
  W is causal-masked rank-2.  Split s/t into 17 chunks of C=126.  For
  target chunk J the contribution of all s < 126J is exactly rank 2:
     out[t in J] = (diag block) + d0^tl * A0_J + w1[t] d1^tl * A1_J
  with A0_J[e] = sum_{s<126J} w0[s] d0^(126J-s) x[s,e]  (A1 analogous).
  C=126 leaves 2 spare K-rows, so the cross term folds into the SAME
  K=128 matmul as the 126x126 diagonal block: two moving-operand
  partitions carry the per-chunk A rows, the rest carry the x chunk.

  v5: all stationaries host-computed (on-device wgen serializes the
  in-order Tensor queue).  Chunk J's A rows live on partition pair
  2*((J-1)//4) so the scatter is 4 contiguous 2-partition HWDGE DMAs.
  Input arrives in 1MB 4-chunk groups on two HWDGE rings with the
  per-chunk A-contribution matmuls (4-strip col-tiled PSUM
  accumulation) hidden under the load.  The strip reduce runs in two
  passes: A_1..A_12 right after chunk 11 (so mains 1..12 overlap the
  input tail), A_13..A_15 after chunk 14, A_16 via a selector matmul
  after chunk 15 — ordered so no matmul ever blocks the in-order
  Tensor queue on a far dependency.  Main bias copies use SEPARATE
  per-half PSUM tensors so Tile's bank tracker doesn't serialize the
  ACT/DVE halves.  Output flushes as 516KB pairs on sync/gpsimd rings.
"""

import numpy as np

import concourse.bass as bass
import concourse.mybir as mybir
import concourse.tile as tile
from concourse import bacc
from concourse.bass_utils import run_bass_kernel_spmd

F16 = mybir.dt.float16
F32 = mybir.dt.float32

B = 8
E = 1024
S = 2048
DC = 1.0
N_CORES = 8
R = (B * E) // N_CORES      # rows (e) per core = 1024
C = 126                     # chunk size along s/t
NCH = 17                    # chunks; chunk 16 has only 32 valid rows
LAST = S - C * (NCH - 1)    # 32

_PROGRAM = None


def _mj(J):
    """A-pair group of chunk J (J>=1): partitions 2m/2m+1 hold its A rows.
    Chunk 0 has no A rows and uses the m=0 layout with zeros there."""
    return (J - 1) // 4 if J >= 1 else 0


def _xrows(J):
    """Partition index for local x row s (s=0..125) of chunk J."""
    m = _mj(J)
    return np.where(np.arange(C) < 2 * m, np.arange(C), np.arange(C) + 2)


def _a2row(J, q):
    """Row of a2q holding A_J component q (J=1..15)."""
    m = (J - 1) // 4
    cnt = 3 if m == 3 else 4
    return 8 * m + q * cnt + (J - 1) % 4


def _build_program():
    nc = bacc.Bacc("TRN2", target_bir_lowering=False, debug=False,
                   num_devices=N_CORES)

    xg_d = nc.declare_dram_parameter("xg", [128, NCH * R], F16, isOutput=False)
    # p1: uu(512) | predq(32) | predb(2) | wf0(126) cols, fp16
    NP1 = 512 + 32 + 2 + C
    p1_d = nc.declare_dram_parameter("p1", [128, NP1], F16, isOutput=False)
    wfa_d = nc.declare_dram_parameter("wfa", [128, 8 * C], F16, isOutput=False)
    wfb_d = nc.declare_dram_parameter("wfb", [128, 8 * C], F16, isOutput=False)
    biasT_d = nc.declare_dram_parameter("biasT", [C, NCH], F32,
                                        isOutput=False)
    outg_d = nc.declare_dram_parameter("outg", [C, NCH * R], F16,
                                       isOutput=True)

    Ident = mybir.ActivationFunctionType.Identity

    with tile.TileContext(nc) as tc:
        with (
            tc.tile_pool(name="cst", bufs=1) as cst,
            tc.tile_pool(name="xp", bufs=1) as xp,
            tc.tile_pool(name="osb", bufs=3) as osb,
            tc.tile_pool(name="ps", bufs=1, space="PSUM") as psp,
            tc.tile_pool(name="p0", bufs=3, space="PSUM") as pop0,
            tc.tile_pool(name="p1", bufs=3, space="PSUM") as pop1,
        ):
            # --- params first on the HWDGE rings, then 1MB x groups
            p1_sb = cst.tile([128, NP1], F16, tag="p1")
            nc.sync.dma_start(p1_sb[:], p1_d[:])
            bias_sb = cst.tile([C, NCH], F32, tag="bias")
            nc.scalar.dma_start(bias_sb[:], biasT_d[:])
            uu_sb = p1_sb[:, 0:512]
            predq_sb = p1_sb[:, 512:544]
            predb_sb = p1_sb[:, 544:546]
            wf0_sb = p1_sb[:, 546:546 + C]

            xg = xp.tile([128, NCH * R], F16, tag="xg")
            groups = [(0, 4, nc.sync), (4, 8, nc.scalar), (8, 12, nc.sync),
                      (12, 16, nc.scalar), (16, 17, nc.sync)]
            for lo, hi, eng in groups:
                eng.dma_start(xg[:, lo * R:hi * R], xg_d[:, lo * R:hi * R])
            wfa_sb = cst.tile([128, 8 * C], F16, tag="wfa")
            nc.scalar.dma_start(wfa_sb[:], wfa_d[:])
            wfb_sb = cst.tile([128, 8 * C], F16, tag="wfb")
            nc.scalar.dma_start(wfb_sb[:], wfb_d[:])

            def wf(J):
                if J == 0:
                    return wf0_sb
                if J <= 8:
                    return wfa_sb[:, (J - 1) * C:J * C]
                return wfb_sb[:, (J - 9) * C:(J - 8) * C]

            def emit_main(J, out_sb, col0, eng_dma=None, dma_src=None,
                          dma_dst=None):
                # separate per-half PSUM tensors: the ACT and DVE copies
                # are independent for Tile's bank tracker and overlap
                poh = [pop0.tile([C, 512], F32, tag="po0", name=f"po{J}h0"),
                       pop1.tile([C, 512], F32, tag="po1", name=f"po{J}h1")]
                for h in range(2):
                    nc.tensor.matmul(poh[h][:], wf(J),
                                     xg[:, R * J + 512 * h:
                                        R * J + 512 * (h + 1)],
                                     start=True, stop=True)
                nc.scalar.activation(out_sb[:, col0:col0 + 512],
                                     poh[0][:], Ident,
                                     bias=bias_sb[:, J:J + 1])
                nc.vector.tensor_scalar_add(out_sb[:, col0 + 512:col0 + 1024],
                                            poh[1][:],
                                            bias_sb[:, J:J + 1])
                if eng_dma is not None:
                    eng_dma.dma_start(dma_dst, dma_src)

            # --- A-phase strips
            a_ps = [psp.tile([128, 512], F32, tag=f"pa{h}", name=f"pa{h}")
                    for h in range(2)]

            def emit_a(I):
                g = I % 4
                for h in range(2):
                    nc.tensor.matmul(a_ps[h][32 * g:32 * (g + 1), :],
                                     uu_sb[:, 32 * I:32 * (I + 1)],
                                     xg[:, R * I + 512 * h:
                                        R * I + 512 * (h + 1)],
                                     start=(I <= 3), stop=(I >= NCH - 5),
                                     skip_group_check=True,
                                     tile_position=(0, 32 * g))

            # chunk 0 main + out0 on the idle gpsimd ring, early
            out0 = osb.tile([C, R], F16, tag="os0")
            emit_main(0, out0, 0, nc.gpsimd, out0[:], outg_d[:, 0:R])

            for I in range(0, 15):        # chunks 0..14 as groups land
                emit_a(I)

            # --- pass A: A_1..A_15 final once chunks 0..14 are in.  Copy
            # strips to SBUF per-strip (each finalizes at a different
            # chunk), reduce with predq, scatter per 4-chunk group.
            a4a_sb = cst.tile([128, R], F16, tag="a4a")
            a2qa_sb = cst.tile([32, R], F16, tag="a2qa")
            for g in range(4):
                sl_ = slice(32 * g, 32 * (g + 1))
                nc.scalar.activation(a4a_sb[sl_, 0:512], a_ps[0][sl_, :],
                                     Ident)
                nc.vector.tensor_copy(a4a_sb[sl_, 512:1024], a_ps[1][sl_, :])
            for h in range(2):
                ar = pop0.tile([32, 512], F32, tag="po0", name=f"ar{h}")
                nc.tensor.matmul(ar[:], predq_sb[:],
                                 a4a_sb[:, 512 * h:512 * (h + 1)],
                                 start=True, stop=True)
                if h == 0:
                    nc.scalar.activation(a2qa_sb[:, 0:512], ar[:], Ident)
                else:
                    nc.vector.tensor_copy(a2qa_sb[:, 512:1024], ar[:])
            nc.sync.dma_start(xg[0:2, R:5 * R], a2qa_sb[0:8, :])
            nc.scalar.dma_start(xg[2:4, 5 * R:9 * R], a2qa_sb[8:16, :])
            nc.sync.dma_start(xg[4:6, 9 * R:13 * R], a2qa_sb[16:24, :])
            nc.scalar.dma_start(xg[6:8, 13 * R:16 * R], a2qa_sb[24:30, :])

            pairs = []
            for p in range(8):
                pairs.append(osb.tile([C, 2 * R], F16, tag="osb",
                                      name=f"pair{p}"))

            def emit_pair(p):
                J0 = 1 + 2 * p
                emit_main(J0, pairs[p], 0)
                eng = nc.gpsimd if p % 2 == 0 else nc.sync
                emit_main(J0 + 1, pairs[p], R, eng,
                          pairs[p][:], outg_d[:, R * J0:R * (J0 + 2)])

            # --- mains 1..4, then chunk 15's strip write + pass B for
            # A_16 (strips 0..2 final in a4a; refresh strip 3 rows 96/97
            # post chunk 15), then mains 5..16.
            emit_pair(0)
            emit_pair(1)
            emit_a(15)
            a2b_sb = cst.tile([2, R], F16, tag="a2b")
            for h in range(2):
                if h == 0:
                    nc.scalar.activation(a4a_sb[96:98, 0:512],
                                         a_ps[0][96:98, :], Ident)
                else:
                    nc.vector.tensor_copy(a4a_sb[96:98, 512:1024],
                                          a_ps[1][96:98, :])
                arb = pop1.tile([2, 512], F32, tag="po1", name=f"arb{h}")
                nc.tensor.matmul(arb[:], predb_sb[:],
                                 a4a_sb[:, 512 * h:512 * (h + 1)],
                                 start=True, stop=True)
                if h == 0:
                    nc.scalar.activation(a2b_sb[:, 0:512], arb[:], Ident)
                else:
                    nc.vector.tensor_copy(a2b_sb[:, 512:1024], arb[:])
            nc.scalar.dma_start(xg[6:8, 16 * R:17 * R], a2b_sb[:])

            for p in range(2, 8):
                emit_pair(p)

    nc.compile()
    return nc


def _host_prep(weight, bias, decay_value):
    w0 = np.zeros(C * NCH); w1 = np.zeros(C * NCH)
    w0[:S] = weight[0].astype(np.float64)
    w1[:S] = weight[1].astype(np.float64)
    d0 = float(np.clip(np.float32(decay_value[0, 0]), 0.9, 1.0))
    d1 = float(np.clip(np.float32(decay_value[1, 0]), 0.9, 1.0))
    sl = np.arange(C, dtype=np.float64)

    uu = np.zeros((128, 16 * 32), dtype=np.float16)
    wfs = np.zeros((NCH, 128, C), dtype=np.float16)
    with np.errstate(under='ignore'):
        for I in range(NCH - 1):
            rows = _xrows(I)
            for J in range(I + 1, NCH):
                e = (126.0 * (J - I) - sl) / DC
                m2 = 2 * (J % 16)       # J=16 -> strip rows 0/1
                uu[rows, 32 * I + m2] = (w0[C * I:C * (I + 1)] * d0 ** e
                                         ).astype(np.float16)
                uu[rows, 32 * I + m2 + 1] = (d1 ** e).astype(np.float16)

        tl = sl
        causal = tl[None, :] >= sl[:, None]
        for J in range(NCH):
            c0 = C * J
            rows = _xrows(J)
            blk = (w0[c0:c0 + C, None] * d0 ** (tl[None, :] - sl[:, None])
                   + w1[None, c0:c0 + C] * d1 ** (tl[None, :] - sl[:, None]))
            blk = np.where(causal, blk, 0.0)
            wfs[J][rows, :] = blk.astype(np.float16)
            if J > 0:
                ap = 2 * _mj(J)
                wfs[J, ap, :] = (d0 ** tl).astype(np.float16)
                wfs[J, ap + 1, :] = (w1[c0:c0 + C] * d1 ** tl
                                     ).astype(np.float16)

    # strip reduce: a2q row _a2row(J,q) = sum_g a4[32g + 2(J%16) + q]
    predq = np.zeros((128, 32), dtype=np.float16)
    for g in range(4):
        for J in range(1, NCH - 1):
            for q in range(2):
                predq[32 * g + 2 * (J % 16) + q, _a2row(J, q)] = 1
    # A_16 selector: sum strip rows 0/1 across the four strips
    predb = np.zeros((128, 2), dtype=np.float16)
    for g in range(4):
        for q in range(2):
            predb[32 * g + q, q] = 1

    p1 = np.zeros((128, 512 + 32 + 2 + C), dtype=np.float16)
    p1[:, 0:512] = uu
    p1[:, 512:544] = predq
    p1[:, 544:546] = predb
    p1[:, 546:546 + C] = wfs[0]
    wfa = np.ascontiguousarray(
        wfs[1:9].transpose(1, 0, 2).reshape(128, 8 * C))
    wfb = np.ascontiguousarray(
        wfs[9:17].transpose(1, 0, 2).reshape(128, 8 * C))

    biasT = np.zeros((C, NCH), dtype=np.float32)
    bias32 = bias.astype(np.float32)
    for J in range(NCH):
        hi = min(C, S - C * J)
        biasT[:hi, J] = bias32[C * J:C * J + hi]
    return p1, wfa, wfb, biasT


def make_in_maps(inputs):
    x = np.asarray(inputs["x"], dtype=np.float32)
    weight = np.asarray(inputs["weight"], dtype=np.float32)
    bias = np.asarray(inputs["bias"], dtype=np.float32)
    decay_value = np.asarray(inputs["decay_value"], dtype=np.float32)

    p1, wfa, wfb, biasT = _host_prep(weight, bias, decay_value)

    x16 = x.reshape(B * E, S).astype(np.float16)
    in_maps = []
    for c in range(N_CORES):
        xc = x16[R * c:R * (c + 1), :]                    # [R, S]
        xgc = np.zeros((128, NCH * R), dtype=np.float16)
        xcT = xc.T                                        # [S, R]
        for J in range(NCH):
            hi = min(C, S - C * J)
            rows = _xrows(J)[:hi]
            xgc[np.ix_(rows, np.arange(R * J, R * (J + 1)))] = \
                xcT[C * J:C * J + hi, :]
        in_maps.append({
            "xg": np.ascontiguousarray(xgc), "p1": p1, "wfa": wfa,
            "wfb": wfb, "biasT": biasT,
        })
    return in_maps


def kernel(x, weight, bias, decay_value, index=0, recurrent=0, **_):
    global _PROGRAM
    if _PROGRAM is None:
        _PROGRAM = _build_program()
    nc = _PROGRAM

    in_maps = make_in_maps({"x": x, "weight": weight, "bias": bias,
                            "decay_value": decay_value})

    res = run_bass_kernel_spmd(nc, in_maps, core_ids=list(range(N_CORES)))
    out = np.empty((B * E, S), dtype=np.float32)
    for c in range(N_CORES):
        og = np.asarray(res.results[c]["outg"])            # [C, NCH*R] f16
        ot = np.empty((S, R), dtype=np.float32)
        for J in range(NCH):
            hi = min(C, S - C * J)
            ot[C * J:C * J + hi, :] = og[0:hi, R * J:R * (J + 1)
                                         ].astype(np.float32)
        out[R * c:R * (c + 1), :] = ot.T
    return out.reshape(B, E, S)


# revision 19
# speedup vs baseline: 1.0915x; 1.0915x over previous
"""Trainium2 Bass kernel for CombinedRepeatCausalLinear (parallel forward).

Computes out[b,e,t] = sum_s x[b,e,s] * W[s,t] + bias[t] where
  W[s,t] = mask(t>=s) * (w0[s]*d0^(t-s) + w1[t]*d1^(t-s))
for S = 2048, x of shape (8, 1024, 2048) fp32.

Strategy (8 NeuronCores, data-parallel over batch; fp16 datapath):
  W is causal-masked rank-2.  Split s/t into 17 chunks of C=126.  For
  target chunk J the contribution of all s < 126J is exactly rank 2:
     out[t in J] = (diag block) + d0^tl * A0_J + w1[t] d1^tl * A1_J
  with A0_J[e] = sum_{s<126J} w0[s] d0^(126J-s) x[s,e]  (A1 analogous).
  C=126 leaves 2 spare K-rows, so the cross term folds into the SAME
  K=128 matmul as the 126x126 diagonal block: two moving-operand
  partitions carry the per-chunk A rows, the rest carry the x chunk.

  v5: all stationaries host-computed (on-device wgen serializes the
  in-order Tensor queue).  Chunk J's A rows live on partition pair
  2*((J-1)//4) so the scatter is 4 contiguous 2-partition HWDGE DMAs.
  Input arrives in 1MB 4-chunk groups on two HWDGE rings with the
  per-chunk A-contribution matmuls (4-strip col-tiled PSUM
  accumulation) hidden under the load.  The strip reduce runs in two
  passes: A_1..A_12 right after chunk 11 (so mains 1..12 overlap the
  input tail), A_13..A_15 after chunk 14, A_16 via a selector matmul
  after chunk 15 — ordered so no matmul ever blocks the in-order
  Tensor queue on a far dependency.  Main bias copies use SEPARATE
  per-half PSUM tensors so Tile's bank tracker doesn't serialize the
  ACT/DVE halves.  Output flushes as 516KB pairs on sync/gpsimd rings.
"""

import numpy as np

import concourse.bass as bass
import concourse.mybir as mybir
import concourse.tile as tile
from concourse import bacc
from concourse.bass_utils import run_bass_kernel_spmd

F16 = mybir.dt.float16
F32 = mybir.dt.float32

B = 8
E = 1024
S = 2048
DC = 1.0
N_CORES = 8
R = (B * E) // N_CORES      # rows (e) per core = 1024
C = 126                     # chunk size along s/t
NCH = 17                    # chunks; chunk 16 has only 32 valid rows
LAST = S - C * (NCH - 1)    # 32

_PROGRAM = None


def _mj(J):
    """A-pair group of chunk J (J>=1): partitions 2m/2m+1 hold its A rows.
    Chunk 0 has no A rows and uses the m=0 layout with zeros there."""
    return (J - 1) // 4 if J >= 1 else 0


def _xrows(J):
    """Partition index for local x row s (s=0..125) of chunk J."""
    m = _mj(J)
    return np.where(np.arange(C) < 2 * m, np.arange(C), np.arange(C) + 2)


def _a2row(J, q):
    """Row of a2q holding A_J component q (J=1..15)."""
    m = (J - 1) // 4
    cnt = 3 if m == 3 else 4
    return 8 * m + q * cnt + (J - 1) % 4


def _build_program():
    nc = bacc.Bacc("TRN2", target_bir_lowering=False, debug=False,
                   num_devices=N_CORES)

    xg_d = nc.declare_dram_parameter("xg", [128, NCH * R], F16, isOutput=False)
    # p1: uu(512) | predq(32) | predb(2) | wf0(126) cols, fp16
    NP1 = 512 + 32 + 2 + C
    p1_d = nc.declare_dram_parameter("p1", [128, NP1], F16, isOutput=False)
    wfa_d = nc.declare_dram_parameter("wfa", [128, 8 * C], F16, isOutput=False)
    wfb_d = nc.declare_dram_parameter("wfb", [128, 8 * C], F16, isOutput=False)
    biasT_d = nc.declare_dram_parameter("biasT", [C, NCH], F32,
                                        isOutput=False)
    outg_d = nc.declare_dram_parameter("outg", [C, NCH * R], F16,
                                       isOutput=True)

    Ident = mybir.ActivationFunctionType.Identity

    with tile.TileContext(nc) as tc:
        with (
            tc.tile_pool(name="cst", bufs=1) as cst,
            tc.tile_pool(name="xp", bufs=1) as xp,
            tc.tile_pool(name="osb", bufs=3) as osb,
            tc.tile_pool(name="ps", bufs=1, space="PSUM") as psp,
            tc.tile_pool(name="p0", bufs=3, space="PSUM") as pop0,
            tc.tile_pool(name="p1", bufs=3, space="PSUM") as pop1,
        ):
            # --- params first on the HWDGE rings, then 1MB x groups
            p1_sb = cst.tile([128, NP1], F16, tag="p1")
            nc.sync.dma_start(p1_sb[:], p1_d[:])
            uu_sb = p1_sb[:, 0:512]
            predq_sb = p1_sb[:, 512:544]
            predb_sb = p1_sb[:, 544:546]
            wf0_sb = p1_sb[:, 546:546 + C]

            # group order balances pre-junction bytes across the two
            # rings so chunks 0..14 (the pass-A gate) all land early;
            # only wfb trails the gate
            xg = xp.tile([128, NCH * R], F16, tag="xg")
            groups = [(0, 4, nc.sync), (4, 8, nc.scalar), (8, 12, nc.scalar),
                      (12, 16, nc.sync), (16, 17, nc.sync)]
            for lo, hi, eng in groups:
                eng.dma_start(xg[:, lo * R:hi * R], xg_d[:, lo * R:hi * R])
            bias_sb = cst.tile([C, NCH], F32, tag="bias")
            nc.scalar.dma_start(bias_sb[:], biasT_d[:])
            wfa_sb = cst.tile([128, 8 * C], F16, tag="wfa")
            nc.scalar.dma_start(wfa_sb[:], wfa_d[:])
            wfb_sb = cst.tile([128, 8 * C], F16, tag="wfb")
            nc.scalar.dma_start(wfb_sb[:], wfb_d[:])

            def wf(J):
                if J == 0:
                    return wf0_sb
                if J <= 8:
                    return wfa_sb[:, (J - 1) * C:J * C]
                return wfb_sb[:, (J - 9) * C:(J - 8) * C]

            def emit_main(J, out_sb, col0, eng_dma=None, dma_src=None,
                          dma_dst=None):
                # separate per-half PSUM tensors: the ACT and DVE copies
                # are independent for Tile's bank tracker and overlap
                poh = [pop0.tile([C, 512], F32, tag="po0", name=f"po{J}h0"),
                       pop1.tile([C, 512], F32, tag="po1", name=f"po{J}h1")]
                for h in range(2):
                    nc.tensor.matmul(poh[h][:], wf(J),
                                     xg[:, R * J + 512 * h:
                                        R * J + 512 * (h + 1)],
                                     start=True, stop=True)
                nc.scalar.activation(out_sb[:, col0:col0 + 512],
                                     poh[0][:], Ident,
                                     bias=bias_sb[:, J:J + 1])
                nc.vector.tensor_scalar_add(out_sb[:, col0 + 512:col0 + 1024],
                                            poh[1][:],
                                            bias_sb[:, J:J + 1])
                if eng_dma is not None:
                    eng_dma.dma_start(dma_dst, dma_src)

            # --- A-phase strips
            a_ps = [psp.tile([128, 512], F32, tag=f"pa{h}", name=f"pa{h}")
                    for h in range(2)]

            def emit_a(I):
                g = I % 4
                for h in range(2):
                    nc.tensor.matmul(a_ps[h][32 * g:32 * (g + 1), :],
                                     uu_sb[:, 32 * I:32 * (I + 1)],
                                     xg[:, R * I + 512 * h:
                                        R * I + 512 * (h + 1)],
                                     start=(I <= 3), stop=(I >= NCH - 5),
                                     skip_group_check=True,
                                     tile_position=(0, 32 * g))

            # chunk 0 main + out0 on the idle gpsimd ring, early
            out0 = osb.tile([C, R], F16, tag="os0")
            emit_main(0, out0, 0, nc.gpsimd, out0[:], outg_d[:, 0:R])

            for I in range(0, 15):        # chunks 0..14 as groups land
                emit_a(I)

            # --- pass A: A_1..A_15 final once chunks 0..14 are in.  Copy
            # strips to SBUF per-strip (each finalizes at a different
            # chunk), reduce with predq, scatter per 4-chunk group.
            a4a_sb = cst.tile([128, R], F16, tag="a4a")
            a2qa_sb = cst.tile([32, R], F16, tag="a2qa")
            nc.scalar.activation(a4a_sb[:, 0:512], a_ps[0][:], Ident)
            nc.vector.tensor_copy(a4a_sb[:, 512:1024], a_ps[1][:])
            for h in range(2):
                ar = pop0.tile([32, 512], F32, tag="po0", name=f"ar{h}")
                nc.tensor.matmul(ar[:], predq_sb[:],
                                 a4a_sb[:, 512 * h:512 * (h + 1)],
                                 start=True, stop=True)
                if h == 0:
                    nc.scalar.activation(a2qa_sb[:, 0:512], ar[:], Ident)
                else:
                    nc.vector.tensor_copy(a2qa_sb[:, 512:1024], ar[:])
            nc.sync.dma_start(xg[0:2, R:5 * R], a2qa_sb[0:8, :])
            nc.scalar.dma_start(xg[2:4, 5 * R:9 * R], a2qa_sb[8:16, :])
            nc.sync.dma_start(xg[4:6, 9 * R:13 * R], a2qa_sb[16:24, :])
            nc.scalar.dma_start(xg[6:8, 13 * R:16 * R], a2qa_sb[24:30, :])

            pairs = []
            for p in range(8):
                pairs.append(osb.tile([C, 2 * R], F16, tag="osb",
                                      name=f"pair{p}"))

            def emit_pair(p):
                J0 = 1 + 2 * p
                emit_main(J0, pairs[p], 0)
                eng = nc.gpsimd if p % 2 == 0 else nc.sync
                emit_main(J0 + 1, pairs[p], R, eng,
                          pairs[p][:], outg_d[:, R * J0:R * (J0 + 2)])

            # --- mains 1..4, then chunk 15's strip write + pass B for
            # A_16 (strips 0..2 final in a4a; refresh strip 3 rows 96/97
            # post chunk 15), then mains 5..16.
            emit_pair(0)
            emit_pair(1)
            emit_a(15)
            a2b_sb = cst.tile([2, R], F16, tag="a2b")
            for h in range(2):
                if h == 0:
                    nc.scalar.activation(a4a_sb[96:98, 0:512],
                                         a_ps[0][96:98, :], Ident)
                else:
                    nc.vector.tensor_copy(a4a_sb[96:98, 512:1024],
                                          a_ps[1][96:98, :])
                arb = pop1.tile([2, 512], F32, tag="po1", name=f"arb{h}")
                nc.tensor.matmul(arb[:], predb_sb[:],
                                 a4a_sb[:, 512 * h:512 * (h + 1)],
                                 start=True, stop=True)
                if h == 0:
                    nc.scalar.activation(a2b_sb[:, 0:512], arb[:], Ident)
                else:
                    nc.vector.tensor_copy(a2b_sb[:, 512:1024], arb[:])
            nc.scalar.dma_start(xg[6:8, 16 * R:17 * R], a2b_sb[:])

            for p in range(2, 8):
                emit_pair(p)

    nc.compile()
    return nc


def _host_prep(weight, bias, decay_value):
    w0 = np.zeros(C * NCH); w1 = np.zeros(C * NCH)
    w0[:S] = weight[0].astype(np.float64)
    w1[:S] = weight[1].astype(np.float64)
    d0 = float(np.clip(np.float32(decay_value[0, 0]), 0.9, 1.0))
    d1 = float(np.clip(np.float32(decay_value[1, 0]), 0.9, 1.0))
    sl = np.arange(C, dtype=np.float64)

    uu = np.zeros((128, 16 * 32), dtype=np.float16)
    wfs = np.zeros((NCH, 128, C), dtype=np.float16)
    with np.errstate(under='ignore'):
        for I in range(NCH - 1):
            rows = _xrows(I)
            for J in range(I + 1, NCH):
                e = (126.0 * (J - I) - sl) / DC
                m2 = 2 * (J % 16)       # J=16 -> strip rows 0/1
                uu[rows, 32 * I + m2] = (w0[C * I:C * (I + 1)] * d0 ** e
                                         ).astype(np.float16)
                uu[rows, 32 * I + m2 + 1] = (d1 ** e).astype(np.float16)

        tl = sl
        causal = tl[None, :] >= sl[:, None]
        for J in range(NCH):
            c0 = C * J
            rows = _xrows(J)
            blk = (w0[c0:c0 + C, None] * d0 ** (tl[None, :] - sl[:, None])
                   + w1[None, c0:c0 + C] * d1 ** (tl[None, :] - sl[:, None]))
            blk = np.where(causal, blk, 0.0)
            wfs[J][rows, :] = blk.astype(np.float16)
            if J > 0:
                ap = 2 * _mj(J)
                wfs[J, ap, :] = (d0 ** tl).astype(np.float16)
                wfs[J, ap + 1, :] = (w1[c0:c0 + C] * d1 ** tl
                                     ).astype(np.float16)

    # strip reduce: a2q row _a2row(J,q) = sum_g a4[32g + 2(J%16) + q]
    predq = np.zeros((128, 32), dtype=np.float16)
    for g in range(4):
        for J in range(1, NCH - 1):
            for q in range(2):
                predq[32 * g + 2 * (J % 16) + q, _a2row(J, q)] = 1
    # A_16 selector: sum strip rows 0/1 across the four strips
    predb = np.zeros((128, 2), dtype=np.float16)
    for g in range(4):
        for q in range(2):
            predb[32 * g + q, q] = 1

    p1 = np.zeros((128, 512 + 32 + 2 + C), dtype=np.float16)
    p1[:, 0:512] = uu
    p1[:, 512:544] = predq
    p1[:, 544:546] = predb
    p1[:, 546:546 + C] = wfs[0]
    wfa = np.ascontiguousarray(
        wfs[1:9].transpose(1, 0, 2).reshape(128, 8 * C))
    wfb = np.ascontiguousarray(
        wfs[9:17].transpose(1, 0, 2).reshape(128, 8 * C))

    biasT = np.zeros((C, NCH), dtype=np.float32)
    bias32 = bias.astype(np.float32)
    for J in range(NCH):
        hi = min(C, S - C * J)
        biasT[:hi, J] = bias32[C * J:C * J + hi]
    return p1, wfa, wfb, biasT


def make_in_maps(inputs):
    x = np.asarray(inputs["x"], dtype=np.float32)
    weight = np.asarray(inputs["weight"], dtype=np.float32)
    bias = np.asarray(inputs["bias"], dtype=np.float32)
    decay_value = np.asarray(inputs["decay_value"], dtype=np.float32)

    p1, wfa, wfb, biasT = _host_prep(weight, bias, decay_value)

    x16 = x.reshape(B * E, S).astype(np.float16)
    in_maps = []
    for c in range(N_CORES):
        xc = x16[R * c:R * (c + 1), :]                    # [R, S]
        xgc = np.zeros((128, NCH * R), dtype=np.float16)
        xcT = xc.T                                        # [S, R]
        for J in range(NCH):
            hi = min(C, S - C * J)
            rows = _xrows(J)[:hi]
            xgc[np.ix_(rows, np.arange(R * J, R * (J + 1)))] = \
                xcT[C * J:C * J + hi, :]
        in_maps.append({
            "xg": np.ascontiguousarray(xgc), "p1": p1, "wfa": wfa,
            "wfb": wfb, "biasT": biasT,
        })
    return in_maps


def kernel(x, weight, bias, decay_value, index=0, recurrent=0, **_):
    global _PROGRAM
    if _PROGRAM is None:
        _PROGRAM = _build_program()
    nc = _PROGRAM

    in_maps = make_in_maps({"x": x, "weight": weight, "bias": bias,
                            "decay_value": decay_value})

    res = run_bass_kernel_spmd(nc, in_maps, core_ids=list(range(N_CORES)))
    out = np.empty((B * E, S), dtype=np.float32)
    for c in range(N_CORES):
        og = np.asarray(res.results[c]["outg"])            # [C, NCH*R] f16
        ot = np.empty((S, R), dtype=np.float32)
        for J in range(NCH):
            hi = min(C, S - C * J)
            ot[C * J:C * J + hi, :] = og[0:hi, R * J:R * (J + 1)
                                         ].astype(np.float32)
        out[R * c:R * (c + 1), :] = ot.T
    return out.reshape(B, E, S)


# revision 27
# speedup vs baseline: 1.0945x; 1.0027x over previous
"""Trainium2 Bass kernel for CombinedRepeatCausalLinear (parallel forward).

Computes out[b,e,t] = sum_s x[b,e,s] * W[s,t] + bias[t] where
  W[s,t] = mask(t>=s) * (w0[s]*d0^(t-s) + w1[t]*d1^(t-s))
for S = 2048, x of shape (8, 1024, 2048) fp32.

Strategy (8 NeuronCores, data-parallel over batch; fp16 datapath):
  W is causal-masked rank-2.  Split s/t into 17 chunks of C=126.  For
  target chunk J the contribution of all s < 126J is exactly rank 2:
     out[t in J] = (diag block) + d0^tl * A0_J + w1[t] d1^tl * A1_J
  with A0_J[e] = sum_{s<126J} w0[s] d0^(126J-s) x[s,e]  (A1 analogous).
  C=126 leaves 2 spare K-rows, so the cross term folds into the SAME
  K=128 matmul as the 126x126 diagonal block: two moving-operand
  partitions carry the per-chunk A rows, the rest carry the x chunk.

  v5: all stationaries host-computed (on-device wgen serializes the
  in-order Tensor queue).  Chunk J's A rows live on partition pair
  2*((J-1)//4) so the scatter is 4 contiguous 2-partition HWDGE DMAs.
  Input arrives in 1MB 4-chunk groups on two HWDGE rings with the
  per-chunk A-contribution matmuls (4-strip col-tiled PSUM
  accumulation) hidden under the load.  The strip reduce runs in two
  passes: A_1..A_12 right after chunk 11 (so mains 1..12 overlap the
  input tail), A_13..A_15 after chunk 14, A_16 via a selector matmul
  after chunk 15 — ordered so no matmul ever blocks the in-order
  Tensor queue on a far dependency.  Main bias copies use SEPARATE
  per-half PSUM tensors so Tile's bank tracker doesn't serialize the
  ACT/DVE halves.  Output flushes as 516KB pairs on sync/gpsimd rings.
"""

import numpy as np

import concourse.bass as bass
import concourse.mybir as mybir
import concourse.tile as tile
from concourse import bacc
from concourse.bass_utils import run_bass_kernel_spmd

F16 = mybir.dt.float16
F32 = mybir.dt.float32

B = 8
E = 1024
S = 2048
DC = 1.0
N_CORES = 8
R = (B * E) // N_CORES      # rows (e) per core = 1024
C = 126                     # chunk size along s/t
NCH = 17                    # chunks; chunk 16 has only 32 valid rows
LAST = S - C * (NCH - 1)    # 32

_PROGRAM = None


def _mj(J):
    """A-pair group of chunk J (J>=1): partitions 2m/2m+1 hold its A rows.
    Chunk 0 has no A rows and uses the m=0 layout with zeros there."""
    return (J - 1) // 4 if J >= 1 else 0


def _xrows(J):
    """Partition index for local x row s (s=0..125) of chunk J."""
    m = _mj(J)
    return np.where(np.arange(C) < 2 * m, np.arange(C), np.arange(C) + 2)


def _a2row(J, q):
    """Row of the A accumulator holding A_J component q (J=1..16).
    Rows are grouped by 4-chunk scatter group, q-major inside; A_16
    sits on rows 30/31 (scattered separately in pass B)."""
    if J == 16:
        return 30 + q
    m = (J - 1) // 4
    cnt = 3 if m == 3 else 4
    return 8 * m + q * cnt + (J - 1) % 4


def _build_program():
    nc = bacc.Bacc("TRN2", target_bir_lowering=False, debug=False,
                   num_devices=N_CORES)

    xg_d = nc.declare_dram_parameter("xg", [128, NCH * R], F16, isOutput=False)
    # p1: uu(512) | wf0(126) cols, fp16
    NP1 = 512 + C
    p1_d = nc.declare_dram_parameter("p1", [128, NP1], F16, isOutput=False)
    wfa_d = nc.declare_dram_parameter("wfa", [128, 8 * C], F16, isOutput=False)
    wfb_d = nc.declare_dram_parameter("wfb", [128, 8 * C], F16, isOutput=False)
    biasT_d = nc.declare_dram_parameter("biasT", [C, NCH], F32,
                                        isOutput=False)
    outg_d = nc.declare_dram_parameter("outg", [C, NCH * R], F16,
                                       isOutput=True)

    Ident = mybir.ActivationFunctionType.Identity

    with tile.TileContext(nc) as tc:
        with (
            tc.tile_pool(name="cst", bufs=1) as cst,
            tc.tile_pool(name="xp", bufs=1) as xp,
            tc.tile_pool(name="osb", bufs=3) as osb,
            tc.tile_pool(name="ps", bufs=1, space="PSUM") as psp,
            tc.tile_pool(name="p0", bufs=3, space="PSUM") as pop0,
            tc.tile_pool(name="p1", bufs=3, space="PSUM") as pop1,
        ):
            # --- params first on the HWDGE rings, then 1MB x groups
            p1_sb = cst.tile([128, NP1], F16, tag="p1")
            nc.sync.dma_start(p1_sb[:], p1_d[:])
            uu_sb = p1_sb[:, 0:512]
            wf0_sb = p1_sb[:, 512:512 + C]

            # group order balances pre-junction bytes across the two
            # rings so chunks 0..14 (the pass-A gate) all land early;
            # only wfb trails the gate
            xg = xp.tile([128, NCH * R], F16, tag="xg")
            groups = [(0, 4, nc.sync), (4, 8, nc.scalar), (8, 12, nc.scalar),
                      (12, 16, nc.sync), (16, 17, nc.sync)]
            for lo, hi, eng in groups:
                eng.dma_start(xg[:, lo * R:hi * R], xg_d[:, lo * R:hi * R])
            bias_sb = cst.tile([C, NCH], F32, tag="bias")
            nc.scalar.dma_start(bias_sb[:], biasT_d[:])
            wfa_sb = cst.tile([128, 8 * C], F16, tag="wfa")
            nc.scalar.dma_start(wfa_sb[:], wfa_d[:])
            wfb_sb = cst.tile([128, 8 * C], F16, tag="wfb")
            nc.scalar.dma_start(wfb_sb[:], wfb_d[:])

            def wf(J):
                if J == 0:
                    return wf0_sb
                if J <= 8:
                    return wfa_sb[:, (J - 1) * C:J * C]
                return wfb_sb[:, (J - 9) * C:(J - 8) * C]

            def emit_main(J, out_sb, col0, eng_dma=None, dma_src=None,
                          dma_dst=None):
                # separate per-half PSUM tensors: the ACT and DVE copies
                # are independent for Tile's bank tracker and overlap
                poh = [pop0.tile([C, 512], F32, tag="po0", name=f"po{J}h0"),
                       pop1.tile([C, 512], F32, tag="po1", name=f"po{J}h1")]
                for h in range(2):
                    nc.tensor.matmul(poh[h][:], wf(J),
                                     xg[:, R * J + 512 * h:
                                        R * J + 512 * (h + 1)],
                                     start=True, stop=True)
                nc.scalar.activation(out_sb[:, col0:col0 + 512],
                                     poh[0][:], Ident,
                                     bias=bias_sb[:, J:J + 1])
                nc.vector.tensor_scalar_add(out_sb[:, col0 + 512:col0 + 1024],
                                            poh[1][:],
                                            bias_sb[:, J:J + 1])
                if eng_dma is not None:
                    eng_dma.dma_start(dma_dst, dma_src)

            # --- A-phase: every chunk accumulates straight into one
            # [32, 512] PSUM bank per half (rows are final A_J rows in
            # scatter order) — no strips, no reduce matmul.  PE has
            # spare capacity under the input load for the serial MMs.
            a_ps = [psp.tile([32, 512], F32, tag=f"pa{h}", name=f"pa{h}")
                    for h in range(2)]

            def emit_a(I):
                for h in range(2):
                    nc.tensor.matmul(a_ps[h][:],
                                     uu_sb[:, 32 * I:32 * (I + 1)],
                                     xg[:, R * I + 512 * h:
                                        R * I + 512 * (h + 1)],
                                     start=(I == 0), stop=(I == NCH - 2))

            # chunk 0 main + out0 on the idle gpsimd ring, early
            out0 = osb.tile([C, R], F16, tag="os0")
            emit_main(0, out0, 0, nc.gpsimd, out0[:], outg_d[:, 0:R])

            for I in range(0, 15):        # chunks 0..14 as groups land
                emit_a(I)

            # --- pass A: A_1..A_15 final once chunks 0..14 are in.
            # Copy the accumulator to SBUF and scatter per 4-chunk group.
            a2qa_sb = cst.tile([32, R], F16, tag="a2qa")
            nc.scalar.activation(a2qa_sb[:, 0:512], a_ps[0][:], Ident)
            nc.vector.tensor_copy(a2qa_sb[:, 512:1024], a_ps[1][:])
            nc.sync.dma_start(xg[0:2, R:5 * R], a2qa_sb[0:8, :])
            nc.scalar.dma_start(xg[2:4, 5 * R:9 * R], a2qa_sb[8:16, :])
            nc.sync.dma_start(xg[4:6, 9 * R:13 * R], a2qa_sb[16:24, :])
            nc.scalar.dma_start(xg[6:8, 13 * R:16 * R], a2qa_sb[24:30, :])

            pairs = []
            for p in range(8):
                pairs.append(osb.tile([C, 2 * R], F16, tag="osb",
                                      name=f"pair{p}"))

            def emit_pair(p):
                J0 = 1 + 2 * p
                emit_main(J0, pairs[p], 0)
                eng = nc.gpsimd if p % 2 == 0 else nc.sync
                emit_main(J0 + 1, pairs[p], R, eng,
                          pairs[p][:], outg_d[:, R * J0:R * (J0 + 2)])

            # --- mains 1..4, then chunk 15's accumulator write + pass B:
            # after chunk 15 the accumulator rows 30/31 hold A_16 — copy
            # the bank again and scatter just those rows.
            emit_pair(0)
            emit_pair(1)
            emit_a(15)
            a2c_sb = cst.tile([32, R], F16, tag="a2c")
            nc.scalar.activation(a2c_sb[:, 0:512], a_ps[0][:], Ident)
            nc.vector.tensor_copy(a2c_sb[:, 512:1024], a_ps[1][:])
            nc.scalar.dma_start(xg[6:8, 16 * R:17 * R], a2c_sb[30:32, :])

            for p in range(2, 8):
                emit_pair(p)

    nc.compile()
    return nc


def _host_prep(weight, bias, decay_value):
    w0 = np.zeros(C * NCH); w1 = np.zeros(C * NCH)
    w0[:S] = weight[0].astype(np.float64)
    w1[:S] = weight[1].astype(np.float64)
    d0 = float(np.clip(np.float32(decay_value[0, 0]), 0.9, 1.0))
    d1 = float(np.clip(np.float32(decay_value[1, 0]), 0.9, 1.0))
    sl = np.arange(C, dtype=np.float64)

    uu = np.zeros((128, 16 * 32), dtype=np.float16)
    wfs = np.zeros((NCH, 128, C), dtype=np.float16)
    with np.errstate(under='ignore'):
        for I in range(NCH - 1):
            rows = _xrows(I)
            for J in range(I + 1, NCH):
                e = (126.0 * (J - I) - sl) / DC
                uu[rows, 32 * I + _a2row(J, 0)] = (
                    w0[C * I:C * (I + 1)] * d0 ** e).astype(np.float16)
                uu[rows, 32 * I + _a2row(J, 1)] = (d1 ** e
                                                   ).astype(np.float16)

        tl = sl
        causal = tl[None, :] >= sl[:, None]
        for J in range(NCH):
            c0 = C * J
            rows = _xrows(J)
            blk = (w0[c0:c0 + C, None] * d0 ** (tl[None, :] - sl[:, None])
                   + w1[None, c0:c0 + C] * d1 ** (tl[None, :] - sl[:, None]))
            blk = np.where(causal, blk, 0.0)
            wfs[J][rows, :] = blk.astype(np.float16)
            if J > 0:
                ap = 2 * _mj(J)
                wfs[J, ap, :] = (d0 ** tl).astype(np.float16)
                wfs[J, ap + 1, :] = (w1[c0:c0 + C] * d1 ** tl
                                     ).astype(np.float16)

    p1 = np.zeros((128, 512 + C), dtype=np.float16)
    p1[:, 0:512] = uu
    p1[:, 512:512 + C] = wfs[0]
    wfa = np.ascontiguousarray(
        wfs[1:9].transpose(1, 0, 2).reshape(128, 8 * C))
    wfb = np.ascontiguousarray(
        wfs[9:17].transpose(1, 0, 2).reshape(128, 8 * C))

    biasT = np.zeros((C, NCH), dtype=np.float32)
    bias32 = bias.astype(np.float32)
    for J in range(NCH):
        hi = min(C, S - C * J)
        biasT[:hi, J] = bias32[C * J:C * J + hi]
    return p1, wfa, wfb, biasT


def make_in_maps(inputs):
    x = np.asarray(inputs["x"], dtype=np.float32)
    weight = np.asarray(inputs["weight"], dtype=np.float32)
    bias = np.asarray(inputs["bias"], dtype=np.float32)
    decay_value = np.asarray(inputs["decay_value"], dtype=np.float32)

    p1, wfa, wfb, biasT = _host_prep(weight, bias, decay_value)

    x16 = x.reshape(B * E, S).astype(np.float16)
    in_maps = []
    for c in range(N_CORES):
        xc = x16[R * c:R * (c + 1), :]                    # [R, S]
        xgc = np.zeros((128, NCH * R), dtype=np.float16)
        xcT = xc.T                                        # [S, R]
        for J in range(NCH):
            hi = min(C, S - C * J)
            rows = _xrows(J)[:hi]
            xgc[np.ix_(rows, np.arange(R * J, R * (J + 1)))] = \
                xcT[C * J:C * J + hi, :]
        in_maps.append({
            "xg": np.ascontiguousarray(xgc), "p1": p1, "wfa": wfa,
            "wfb": wfb, "biasT": biasT,
        })
    return in_maps


def kernel(x, weight, bias, decay_value, index=0, recurrent=0, **_):
    global _PROGRAM
    if _PROGRAM is None:
        _PROGRAM = _build_program()
    nc = _PROGRAM

    in_maps = make_in_maps({"x": x, "weight": weight, "bias": bias,
                            "decay_value": decay_value})

    res = run_bass_kernel_spmd(nc, in_maps, core_ids=list(range(N_CORES)))
    out = np.empty((B * E, S), dtype=np.float32)
    for c in range(N_CORES):
        og = np.asarray(res.results[c]["outg"])            # [C, NCH*R] f16
        ot = np.empty((S, R), dtype=np.float32)
        for J in range(NCH):
            hi = min(C, S - C * J)
            ot[C * J:C * J + hi, :] = og[0:hi, R * J:R * (J + 1)
                                         ].astype(np.float32)
        out[R * c:R * (c + 1), :] = ot.T
    return out.reshape(B, E, S)


# revision 29
# speedup vs baseline: 1.1200x; 1.0233x over previous
"""Trainium2 Bass kernel for CombinedRepeatCausalLinear (parallel forward).

Computes out[b,e,t] = sum_s x[b,e,s] * W[s,t] + bias[t] where
  W[s,t] = mask(t>=s) * (w0[s]*d0^(t-s) + w1[t]*d1^(t-s))
for S = 2048, x of shape (8, 1024, 2048) fp32.

Strategy (8 NeuronCores, data-parallel over batch; fp16 datapath):
  W is causal-masked rank-2.  Split s/t into 17 chunks of C=126.  For
  target chunk J the contribution of all s < 126J is exactly rank 2:
     out[t in J] = (diag block) + d0^tl * A0_J + w1[t] d1^tl * A1_J
  with A0_J[e] = sum_{s<126J} w0[s] d0^(126J-s) x[s,e]  (A1 analogous).
  C=126 leaves 2 spare K-rows, so the cross term folds into the SAME
  K=128 matmul as the 126x126 diagonal block: two moving-operand
  partitions carry the per-chunk A rows, the rest carry the x chunk.

  v5: all stationaries host-computed (on-device wgen serializes the
  in-order Tensor queue).  Chunk J's A rows live on partition pair
  2*((J-1)//4) so the scatter is 4 contiguous 2-partition HWDGE DMAs.
  Input arrives in 1MB 4-chunk groups on two HWDGE rings with the
  per-chunk A-contribution matmuls (4-strip col-tiled PSUM
  accumulation) hidden under the load.  The strip reduce runs in two
  passes: A_1..A_12 right after chunk 11 (so mains 1..12 overlap the
  input tail), A_13..A_15 after chunk 14, A_16 via a selector matmul
  after chunk 15 — ordered so no matmul ever blocks the in-order
  Tensor queue on a far dependency.  Main bias copies use SEPARATE
  per-half PSUM tensors so Tile's bank tracker doesn't serialize the
  ACT/DVE halves.  Output flushes as 516KB pairs on sync/gpsimd rings.
"""

import numpy as np

import concourse.bass as bass
import concourse.mybir as mybir
import concourse.tile as tile
from concourse import bacc
from concourse.bass_utils import run_bass_kernel_spmd

F16 = mybir.dt.float16
F32 = mybir.dt.float32

B = 8
E = 1024
S = 2048
DC = 1.0
N_CORES = 8
R = (B * E) // N_CORES      # rows (e) per core = 1024
C = 126                     # chunk size along s/t
NCH = 17                    # chunks; chunk 16 has only 32 valid rows
LAST = S - C * (NCH - 1)    # 32

_PROGRAM = None


def _mj(J):
    """A-pair group of chunk J (J>=1): partitions 2m/2m+1 hold its A rows.
    Chunk 0 has no A rows and uses the m=0 layout with zeros there."""
    return (J - 1) // 4 if J >= 1 else 0


def _xrows(J):
    """Partition index for local x row s (s=0..125) of chunk J."""
    m = _mj(J)
    return np.where(np.arange(C) < 2 * m, np.arange(C), np.arange(C) + 2)


def _a2row(J, q):
    """Row of the A accumulator holding A_J component q (J=1..16).
    Rows are grouped by 4-chunk scatter group, q-major inside; A_16
    sits on rows 30/31 (scattered separately in pass B)."""
    if J == 16:
        return 30 + q
    m = (J - 1) // 4
    cnt = 3 if m == 3 else 4
    return 8 * m + q * cnt + (J - 1) % 4


def _build_program():
    nc = bacc.Bacc("TRN2", target_bir_lowering=False, debug=False,
                   num_devices=N_CORES)

    xg_d = nc.declare_dram_parameter("xg", [128, NCH * R], F16, isOutput=False)
    # p1: uu(512) | wf0(126) cols, fp16
    NP1 = 512 + C
    p1_d = nc.declare_dram_parameter("p1", [128, NP1], F16, isOutput=False)
    wfa_d = nc.declare_dram_parameter("wfa", [128, 8 * C], F16, isOutput=False)
    wfb_d = nc.declare_dram_parameter("wfb", [128, 8 * C], F16, isOutput=False)
    biasT_d = nc.declare_dram_parameter("biasT", [C, NCH], F32,
                                        isOutput=False)
    outg_d = nc.declare_dram_parameter("outg", [C, NCH * R], F16,
                                       isOutput=True)

    Ident = mybir.ActivationFunctionType.Identity

    with tile.TileContext(nc) as tc:
        with (
            tc.tile_pool(name="cst", bufs=1) as cst,
            tc.tile_pool(name="xp", bufs=1) as xp,
            tc.tile_pool(name="osb", bufs=3) as osb,
            tc.tile_pool(name="ps", bufs=1, space="PSUM") as psp,
            tc.tile_pool(name="p0", bufs=3, space="PSUM") as pop0,
            tc.tile_pool(name="p1", bufs=3, space="PSUM") as pop1,
        ):
            # --- params first on the HWDGE rings, then 1MB x groups
            p1_sb = cst.tile([128, NP1], F16, tag="p1")
            nc.sync.dma_start(p1_sb[:], p1_d[:])
            uu_sb = p1_sb[:, 0:512]
            wf0_sb = p1_sb[:, 512:512 + C]

            # x arrives as chunk PAIRS alternating rings: with packet
            # round-robin each ring runs at ~half rate, so adjacent pairs
            # on opposite rings complete in chunk order — the serial
            # A-accumulation chain is fed just in time.  wfa rides early
            # on scalar; wfb and chunk 16 trail (needed late).
            bias_sb = cst.tile([C, NCH], F32, tag="bias")
            nc.scalar.dma_start(bias_sb[:], biasT_d[:])
            xg = xp.tile([128, NCH * R], F16, tag="xg")
            wfa_sb = cst.tile([128, 8 * C], F16, tag="wfa")
            wfb_sb = cst.tile([128, 8 * C], F16, tag="wfb")
            for p in range(8):
                lo, hi = 2 * p, 2 * p + 2
                eng = nc.sync if p % 2 == 0 else nc.scalar
                eng.dma_start(xg[:, lo * R:hi * R], xg_d[:, lo * R:hi * R])
                if p == 3:
                    nc.scalar.dma_start(wfa_sb[:], wfa_d[:])
            nc.sync.dma_start(xg[:, 16 * R:17 * R], xg_d[:, 16 * R:17 * R])
            nc.scalar.dma_start(wfb_sb[:], wfb_d[:])

            def wf(J):
                if J == 0:
                    return wf0_sb
                if J <= 8:
                    return wfa_sb[:, (J - 1) * C:J * C]
                return wfb_sb[:, (J - 9) * C:(J - 8) * C]

            def emit_main(J, out_sb, col0, eng_dma=None, dma_src=None,
                          dma_dst=None):
                # separate per-half PSUM tensors: the ACT and DVE copies
                # are independent for Tile's bank tracker and overlap
                poh = [pop0.tile([C, 512], F32, tag="po0", name=f"po{J}h0"),
                       pop1.tile([C, 512], F32, tag="po1", name=f"po{J}h1")]
                for h in range(2):
                    nc.tensor.matmul(poh[h][:], wf(J),
                                     xg[:, R * J + 512 * h:
                                        R * J + 512 * (h + 1)],
                                     start=True, stop=True)
                nc.scalar.activation(out_sb[:, col0:col0 + 512],
                                     poh[0][:], Ident,
                                     bias=bias_sb[:, J:J + 1])
                nc.vector.tensor_scalar_add(out_sb[:, col0 + 512:col0 + 1024],
                                            poh[1][:],
                                            bias_sb[:, J:J + 1])
                if eng_dma is not None:
                    eng_dma.dma_start(dma_dst, dma_src)

            # --- A-phase: every chunk accumulates straight into one
            # [32, 512] PSUM bank per half (rows are final A_J rows in
            # scatter order) — no strips, no reduce matmul.  PE has
            # spare capacity under the input load for the serial MMs.
            a_ps = [psp.tile([32, 512], F32, tag=f"pa{h}", name=f"pa{h}")
                    for h in range(2)]

            def emit_a(I):
                for h in range(2):
                    nc.tensor.matmul(a_ps[h][:],
                                     uu_sb[:, 32 * I:32 * (I + 1)],
                                     xg[:, R * I + 512 * h:
                                        R * I + 512 * (h + 1)],
                                     start=(I == 0), stop=(I == NCH - 2))

            # chunk 0 main + out0 on the gpsimd ring, early
            out0 = osb.tile([C, R], F16, tag="os0")
            emit_main(0, out0, 0, nc.gpsimd, out0[:], outg_d[:, 0:R])

            # --- A-phase with fine-grained scatter: A_1..A_8 are final
            # after chunk 7, A_9..A_15 after chunk 14, A_16 after chunk
            # 15.  Snapshot the accumulator at those points and scatter
            # on the gpsimd ring — all hidden under the input stream.
            def emit_snap(name, dmas):
                snap = cst.tile([32, R], F16, tag=name, name=name)
                nc.scalar.activation(snap[:, 0:512], a_ps[0][:], Ident)
                nc.vector.tensor_copy(snap[:, 512:1024], a_ps[1][:])
                for dst, rows in dmas:
                    nc.gpsimd.dma_start(dst, snap[rows[0]:rows[1], :])

            for I in range(0, 8):
                emit_a(I)
            emit_snap("a2e", [(xg[0:2, R:5 * R], (0, 8)),
                              (xg[2:4, 5 * R:9 * R], (8, 16))])
            for I in range(8, 15):
                emit_a(I)
            emit_snap("a2l", [(xg[4:6, 9 * R:13 * R], (16, 24)),
                              (xg[6:8, 13 * R:16 * R], (24, 30))])
            emit_a(15)
            emit_snap("a2c", [(xg[6:8, 16 * R:17 * R], (30, 32))])

            pairs = []
            for p in range(8):
                pairs.append(osb.tile([C, 2 * R], F16, tag="osb",
                                      name=f"pair{p}"))

            def emit_pair(p):
                J0 = 1 + 2 * p
                emit_main(J0, pairs[p], 0)
                eng = nc.sync if p % 2 == 0 else nc.scalar
                emit_main(J0 + 1, pairs[p], R, eng,
                          pairs[p][:], outg_d[:, R * J0:R * (J0 + 2)])

            for p in range(8):
                emit_pair(p)

    nc.compile()
    return nc


def _host_prep(weight, bias, decay_value):
    w0 = np.zeros(C * NCH); w1 = np.zeros(C * NCH)
    w0[:S] = weight[0].astype(np.float64)
    w1[:S] = weight[1].astype(np.float64)
    d0 = float(np.clip(np.float32(decay_value[0, 0]), 0.9, 1.0))
    d1 = float(np.clip(np.float32(decay_value[1, 0]), 0.9, 1.0))
    sl = np.arange(C, dtype=np.float64)

    uu = np.zeros((128, 16 * 32), dtype=np.float16)
    wfs = np.zeros((NCH, 128, C), dtype=np.float16)
    with np.errstate(under='ignore'):
        for I in range(NCH - 1):
            rows = _xrows(I)
            for J in range(I + 1, NCH):
                e = (126.0 * (J - I) - sl) / DC
                uu[rows, 32 * I + _a2row(J, 0)] = (
                    w0[C * I:C * (I + 1)] * d0 ** e).astype(np.float16)
                uu[rows, 32 * I + _a2row(J, 1)] = (d1 ** e
                                                   ).astype(np.float16)

        tl = sl
        causal = tl[None, :] >= sl[:, None]
        for J in range(NCH):
            c0 = C * J
            rows = _xrows(J)
            blk = (w0[c0:c0 + C, None] * d0 ** (tl[None, :] - sl[:, None])
                   + w1[None, c0:c0 + C] * d1 ** (tl[None, :] - sl[:, None]))
            blk = np.where(causal, blk, 0.0)
            wfs[J][rows, :] = blk.astype(np.float16)
            if J > 0:
                ap = 2 * _mj(J)
                wfs[J, ap, :] = (d0 ** tl).astype(np.float16)
                wfs[J, ap + 1, :] = (w1[c0:c0 + C] * d1 ** tl
                                     ).astype(np.float16)

    p1 = np.zeros((128, 512 + C), dtype=np.float16)
    p1[:, 0:512] = uu
    p1[:, 512:512 + C] = wfs[0]
    wfa = np.ascontiguousarray(
        wfs[1:9].transpose(1, 0, 2).reshape(128, 8 * C))
    wfb = np.ascontiguousarray(
        wfs[9:17].transpose(1, 0, 2).reshape(128, 8 * C))

    biasT = np.zeros((C, NCH), dtype=np.float32)
    bias32 = bias.astype(np.float32)
    for J in range(NCH):
        hi = min(C, S - C * J)
        biasT[:hi, J] = bias32[C * J:C * J + hi]
    return p1, wfa, wfb, biasT


def make_in_maps(inputs):
    x = np.asarray(inputs["x"], dtype=np.float32)
    weight = np.asarray(inputs["weight"], dtype=np.float32)
    bias = np.asarray(inputs["bias"], dtype=np.float32)
    decay_value = np.asarray(inputs["decay_value"], dtype=np.float32)

    p1, wfa, wfb, biasT = _host_prep(weight, bias, decay_value)

    x16 = x.reshape(B * E, S).astype(np.float16)
    in_maps = []
    for c in range(N_CORES):
        xc = x16[R * c:R * (c + 1), :]                    # [R, S]
        xgc = np.zeros((128, NCH * R), dtype=np.float16)
        xcT = xc.T                                        # [S, R]
        for J in range(NCH):
            hi = min(C, S - C * J)
            rows = _xrows(J)[:hi]
            xgc[np.ix_(rows, np.arange(R * J, R * (J + 1)))] = \
                xcT[C * J:C * J + hi, :]
        in_maps.append({
            "xg": np.ascontiguousarray(xgc), "p1": p1, "wfa": wfa,
            "wfb": wfb, "biasT": biasT,
        })
    return in_maps


def kernel(x, weight, bias, decay_value, index=0, recurrent=0, **_):
    global _PROGRAM
    if _PROGRAM is None:
        _PROGRAM = _build_program()
    nc = _PROGRAM

    in_maps = make_in_maps({"x": x, "weight": weight, "bias": bias,
                            "decay_value": decay_value})

    res = run_bass_kernel_spmd(nc, in_maps, core_ids=list(range(N_CORES)))
    out = np.empty((B * E, S), dtype=np.float32)
    for c in range(N_CORES):
        og = np.asarray(res.results[c]["outg"])            # [C, NCH*R] f16
        ot = np.empty((S, R), dtype=np.float32)
        for J in range(NCH):
            hi = min(C, S - C * J)
            ot[C * J:C * J + hi, :] = og[0:hi, R * J:R * (J + 1)
                                         ].astype(np.float32)
        out[R * c:R * (c + 1), :] = ot.T
    return out.reshape(B, E, S)
